# revision 7
# baseline (speedup 1.0000x reference)
"""Trainium2 Bass kernel for AttentionAggregationNN (ragged single-query MHA pooling).

Algebraic reduction: with one shared query vector, softmax-attention pooling per
group collapses to, per instance i and head h:
    e[i,h]   = exp(s_h . x_i)          (softmax shift-invariance drops the
                                        constant logit term)
    val[i,h] = t_h . x_i
    out[g]   = sum_h (sum_{i in g} e*val) / (sum_{i in g} e) + CONST
where s_h = Wk_h^T q_h / sqrt(D), t_h = Wv_h^T (w_lin @ w_out)_h, folded on the
host in float64.

Device work per core (data-parallel over groups, host pre-sorts by group):
  mm1: SP[128,16] = X_subtile.T @ W16 in ONE fp8 DoubleRow matmul per subtile
       (contract dim 256 = 2 k-tiles of 128; X and W16 both fp8e4m3, W16
       prescaled by power-of-2 per column class, descaled via the ACT input
       scale for scores and the output tensor_scalar for vals)
  ACT: e = exp(scores * 1/sc_s);  DVE: ev = e * vals
  one-hot M[i,j] = (rel_gid[i] == j)              (one DVE op per batch)
  mm2: acc[16, win] += [e|ev].T @ M               (segment sums into PSUM,
                                                   3 PE column-group strips)
  epilogue per half: bf16 PE transpose of the accumulator, strip-sum via two
  DVE adds on [128,16], per-head reciprocal+mult on [128,8] (full DVE lanes),
  head-reduce, [128,1]->[1,128] transpose-matmul, fused *1/sc_t + CONST, and
  a per-half [1,128] output DMA (half0's rides under the stream).

DMA schedule: X is interleaved [P, 2, rows] in DRAM so ONE dma_start fetches
both E-halves of a macro-tile; macros alternate between the sync and scalar
rings.  This keeps the total hw-dma_start count low (the Tile scheduler
rotates just 8 completion semaphores — more dma_starts stall the rings on
semaphore reuse) while the ~2048-col macro size keeps the PE's wait
granularity fine enough to track the stream.
"""
import os

if os.environ.get("AXON_H4_ENABLED") == "1" or os.environ.get("AXON_TERMINAL_JOB_NAME"):
    plats = os.environ.get("JAX_PLATFORMS", "")
    if "axon" not in plats:
        os.environ["JAX_PLATFORMS"] = "axon,cpu"

import numpy as np

# ---------------------------------------------------------------- problem dims
N, G, E, H, D = 131072, 2048, 256, 8, 32
NCORES = 8
GC = G // NCORES        # 256 groups per core
HC = GC // 2            # 128 groups per half
P = 128                 # partition dim / subtile rows
BATCH = 16              # max subtiles per ACT/DVE batch
HALF_QUANT = 128        # row padding quantum per half (= P)
MACRO = 2048            # macro-tile cols
LOOKAHEAD = 4           # macros of DMA descriptor-gen lookahead

DOUBLE_ROW = True       # fp8 DoubleRow mm1 (one matmul per subtile)

_CACHE: dict = {}


# ---------------------------------------------------------------- host algebra
def _fold_params(query, w_in, b_in, w_out, b_out, w_lin, b_lin):
    q64 = query.reshape(E).astype(np.float64)
    w64, b64 = w_in.astype(np.float64), b_in.astype(np.float64)
    wq, wk, wv = w64[:E], w64[E:2 * E], w64[2 * E:]
    bq, bk, bv = b64[:E], b64[E:2 * E], b64[2 * E:]
    q = wq @ q64 + bq
    qh = q.reshape(H, D)
    S = np.einsum("hde,hd->he", wk.reshape(H, D, E), qh) / np.sqrt(D)
    u = (w_lin.astype(np.float64) @ w_out.astype(np.float64)).reshape(E)
    uh = u.reshape(H, D)
    T = np.einsum("hde,hd->he", wv.reshape(H, D, E), uh)
    const = float(np.einsum("hd,hd->", uh, bv.reshape(H, D))
                  + w_lin.astype(np.float64).reshape(E) @ b_out.astype(np.float64)
                  + b_lin.astype(np.float64)[0])
    W16 = np.concatenate([S.T, T.T], axis=1)    # [E, 16]: cols 0:8 scores, 8:16 vals
    return W16, const


def _shard_prep(tree_preds, group_ids, x_dtype, np_dtype):
    sizes = np.bincount(group_ids, minlength=G)
    offsets = np.concatenate([[0], np.cumsum(sizes)]).astype(np.int64)
    sorter = np.argsort(group_ids, kind="stable")
    Xs = np.ascontiguousarray(tree_preds[sorter])
    gs = group_ids[sorter].astype(np.int64)

    # per (core, half): row range and size
    hstart = offsets[(np.arange(2 * NCORES) * HC)]
    hend = offsets[(np.arange(2 * NCORES) + 1) * HC]
    hrows = (hend - hstart).reshape(NCORES, 2)
    caps = [int(np.ceil(hrows[:, h].max() / HALF_QUANT) * HALF_QUANT) for h in (0, 1)]
    rows_cap = caps[0] + caps[1]
    nsub = rows_cap // P
    nsub_half = [caps[0] // P, caps[1] // P]

    # relative gid within half, -1 for pad rows
    grel = np.full((NCORES, rows_cap), -1, np.int64)
    for c in range(NCORES):
        for h in (0, 1):
            i = 2 * c + h
            n = hend[i] - hstart[i]
            col0 = h * caps[0]
            grel[c, col0:col0 + n] = gs[hstart[i]:hend[i]] - (c * GC + h * HC)
    gsub = grel.reshape(NCORES, nsub, P)
    lo = np.where(gsub >= 0, gsub, G).min(axis=(0, 2))
    hi = np.where(gsub >= 0, gsub, -1).max(axis=(0, 2))
    span = int(np.where(hi >= 0, hi - np.minimum(lo, hi) + 1, 1).max())
    win = 32 if span <= 28 else (64 if span <= 60 else 128)
    assert span <= win, f"one-hot span {span} > {win}"
    woff = np.minimum(np.where(lo < G, lo, 0), HC - win).astype(np.int64)
    assert ((hi < woff + win) | (hi < 0)).all()

    rel = np.where(gsub >= 0, gsub - woff[None, :, None], win).astype(np.float32)
    assert ((rel >= 0) & (rel <= win)).all()
    RELT = np.ascontiguousarray(rel.transpose(0, 2, 1)).astype(np_dtype)  # [NC,P,nsub]

    # interleaved X: XT[c, p, k, col] = E-half k of instance col (k-tiles for
    # the DoubleRow contraction)
    XT = np.zeros((NCORES, P, 2, rows_cap), x_dtype)
    for c in range(NCORES):
        for h in (0, 1):
            i = 2 * c + h
            n = hend[i] - hstart[i]
            col0 = h * caps[0]
            blk = Xs[hstart[i]:hend[i]].T.astype(x_dtype)
            XT[c, :, 0, col0:col0 + n] = blk[:P]
            XT[c, :, 1, col0:col0 + n] = blk[P:]
    return XT, RELT, woff, caps, nsub_half, win


def _macro_schedule(cap, first_half, last_half):
    """One dma_start per macro (both E-halves interleaved). First two macros
    of half0 small to prime the pipe quickly; last macro of half1 small to
    shorten the tail."""
    head = [512, 512] if first_half else []
    tail = [1024] if last_half else []
    mid = cap - sum(head) - sum(tail)
    assert mid >= 0, f"cap {cap} too small for schedule"
    sizes = head + [MACRO] * (mid // MACRO)
    if mid % MACRO:
        sizes.append(mid % MACRO)
    sizes += tail
    assert sum(sizes) == cap and all(s % HALF_QUANT == 0 for s in sizes)
    return sizes


# ---------------------------------------------------------------- bass program
def _build_program(caps, nsub_half, woff, const, win, sc_s, sc_t):
    import concourse.bass as bass
    import concourse.tile as tile
    from concourse import bacc, mybir

    DT = mybir.dt.bfloat16
    XDT = mybir.dt.float8e4
    F32 = mybir.dt.float32
    Exp = mybir.ActivationFunctionType.Exp
    Alu = mybir.AluOpType
    rows_cap = caps[0] + caps[1]
    nsub = nsub_half[0] + nsub_half[1]
    JW = BATCH * win

    nc = bacc.Bacc(None, target_bir_lowering=False)
    xt = nc.dram_tensor("xt", [P, 2, rows_cap], XDT, kind="ExternalInput")
    w8t = nc.dram_tensor("w8t", [P, 32], XDT, kind="ExternalInput")
    cbt = nc.dram_tensor("cbt", [P, nsub], DT, kind="ExternalInput")
    out = nc.dram_tensor("out", [1, GC], F32, kind="ExternalOutput")

    # macro schedule, flat across both halves; macro i rides ring i%2
    macros = []
    for h in (0, 1):
        m0 = 0 if h == 0 else caps[0]
        for msz in _macro_schedule(caps[h], first_half=(h == 0),
                                   last_half=(h == 1)):
            macros.append((h, m0, msz))
            m0 += msz
    n_macros = len(macros)

    with tile.TileContext(nc) as tc:
        with (
            tc.tile_pool(name="const", bufs=1) as constp,
            tc.tile_pool(name="xp", bufs=n_macros) as xp,
            tc.tile_pool(name="work", bufs=4) as workp,
            tc.tile_pool(name="ep", bufs=1) as epsb,
            tc.tile_pool(name="mm1", bufs=5, space="PSUM") as mm1p,
            tc.tile_pool(name="acc", bufs=1, space="PSUM") as accp,
            tc.tile_pool(name="tps", bufs=1, space="PSUM") as tpsp,
        ):
            xi_t = {}

            def issue(i):
                if i >= n_macros:
                    return
                _, im0, imsz = macros[i]
                xi_t[i] = xp.tile([P, 2, MACRO], XDT, tag="xi", name=f"xi_{i}")
                ring = nc.scalar if i % 2 == 0 else nc.sync
                ring.dma_start(xi_t[i][:, :, 0:imsz], xt[:, :, im0:im0 + imsz])

            # ---- ring priming: w8 on sync, macro0 on scalar, then relt and
            # the next few macros.
            w8 = constp.tile([P, 32], XDT)
            nc.sync.dma_start(w8[:], w8t[:])
            w8v = w8[:].rearrange("p (k c) -> p k c", c=16)
            issue(0)
            cb_t = constp.tile([P, nsub], DT)
            nc.scalar.dma_start(cb_t[:], cbt[:])
            for i in range(1, LOOKAHEAD):
                issue(i)

            # ---- device-generated constants (gpsimd/vector; rings stay free)
            # j-iota [P, JW]: value j % win, identical on every partition
            jt = constp.tile([P, JW], DT)
            nc.gpsimd.iota(jt[:], [[0, BATCH], [1, win]], channel_multiplier=0,
                           allow_small_or_imprecise_dtypes=True)
            # identity [P, P] bf16 for the PE transposes, via (row == col)
            pi_t = constp.tile([P, 1], F32)
            nc.gpsimd.iota(pi_t[:], [[0, 1]], channel_multiplier=1,
                           allow_small_or_imprecise_dtypes=True)
            jr_t = constp.tile([P, P], F32)
            nc.gpsimd.iota(jr_t[:], [[1, P]], channel_multiplier=0,
                           allow_small_or_imprecise_dtypes=True)
            ident = constp.tile([P, P], DT)
            nc.vector.tensor_tensor(ident[:], jr_t[:],
                                    pi_t[:].to_broadcast([P, P]),
                                    op=Alu.is_equal)
            # zeros for the PSUM-clearing matmul
            zw = constp.tile([P, 2 * HC], DT)
            nc.vector.memset(zw[:], 0.0)

            # Three accumulator strips per half at partition bases 0/32/64
            # (quadrant 3 is unusable per HW erratum): subtile s accumulates
            # into strip s%3; the inferred tile_position packs the mm2s into
            # disjoint PE column groups so they run concurrently.
            accB = accp.tile([P, 2 * HC], F32, tag="accB", name="accB")
            accs = [accB[:, 0:HC], accB[:, HC:2 * HC]]
            nc.tensor.matmul(accB[:, 0:2 * HC], lhsT=zw[:, 0:P], rhs=zw[:],
                             start=True, stop=False, skip_group_check=True)

            oo2 = epsb.tile([1, GC], F32, tag="oo2")

            def epilogue(h):
                # acc -> bf16 -> PE transpose: tpb[g, 32k+c]
                cc = epsb.tile([P, HC], DT, tag=f"cc{h}")
                nc.vector.tensor_copy(cc[:], accs[h])
                tpb = tpsp.tile([P, P], DT, tag="tpb")
                nc.tensor.transpose(tpb[:], cc[:], ident[:])
                # strip block to SBUF (DVE reads at most one PSUM input)
                ts_ = epsb.tile([P, 80], DT, tag=f"ts{h}")
                nc.vector.tensor_copy(ts_[:], tpb[:, 0:80])
                # strip-sum on the free axis: es[:, 0:8] e-sums, 8:16 ev-sums
                e01 = epsb.tile([P, 16], DT, tag=f"e01{h}")
                nc.vector.tensor_tensor(e01[:], ts_[:, 0:16], ts_[:, 32:48],
                                        op=Alu.add)
                es = epsb.tile([P, 16], DT, tag=f"es{h}")
                nc.vector.tensor_tensor(es[:], e01[:], ts_[:, 64:80], op=Alu.add)
                rec = epsb.tile([P, 8], F32, tag=f"rec{h}")
                nc.vector.reciprocal(rec[:], es[:, 0:8])
                rr = epsb.tile([P, 8], DT, tag=f"rr{h}")
                nc.vector.tensor_tensor(rr[:], rec[:], es[:, 8:16], op=Alu.mult)
                oo1 = epsb.tile([P, 1], DT, tag=f"oo1{h}")
                with nc.allow_low_precision("8-term head-sum; bf16 ample here"):
                    nc.vector.tensor_reduce(oo1[:], rr[:],
                                            axis=mybir.AxisListType.X, op=Alu.add)
                # [128,1] -> [1,128] via matmul with identity, then fused
                # val-descale + const into this half's slot of oo2, and a
                # per-half output DMA (half0's rides under the stream).
                opst = tpsp.tile([1, P], F32, tag="opst")
                nc.tensor.matmul(opst[:], lhsT=oo1[:], rhs=ident[:],
                                 start=True, stop=True, skip_group_check=True)
                nc.vector.tensor_scalar(oo2[0:1, h * HC:(h + 1) * HC],
                                        opst[:], 1.0 / sc_t, float(const),
                                        op0=Alu.mult, op1=Alu.add)
                nc.sync.dma_start(out[0:1, h * HC:(h + 1) * HC],
                                  oo2[0:1, h * HC:(h + 1) * HC])

            # ---- main loop. mm2s are emitted one batch late so the PE never
            # stalls on the ACT->DVE chain of the batch it just produced.
            pending = None          # (sp2, m_t, s0, bsz, h)

            ranges = [(0, nsub_half[0]), (nsub_half[0], nsub_half[0] + nsub_half[1])]
            last_q = [{k: max(s for s in range(b, e) if s % 3 == k)
                       for k in range(3)} for (b, e) in ranges]

            def flush_pending():
                nonlocal pending
                if pending is None:
                    return
                sp2, m_t, s0, bsz, ph = pending
                acc = accs[ph]
                for j in range(bsz):
                    s_i = s0 + j
                    k = s_i % 3
                    nc.tensor.matmul(
                        acc[32 * k:32 * k + 16, woff[s_i]:woff[s_i] + win],
                        lhsT=sp2[:, j * 16:j * 16 + 16],
                        rhs=m_t[:, j * win:(j + 1) * win],
                        start=False, stop=(s_i == last_q[ph][k]),
                        skip_group_check=True)
                pending = None

            s = 0
            cur_h = 0
            for mi, (h, m0, msz) in enumerate(macros):
                if h != cur_h:
                    flush_pending()
                    epilogue(cur_h)
                    cur_h = h
                issue(mi + LOOKAHEAD)
                xi = xi_t.pop(mi)
                b0 = 0
                while b0 < msz:
                    bsz = min(BATCH, (msz - b0) // P)      # subtiles in batch
                    spp = mm1p.tile([P, 16 * BATCH], F32)
                    m_t = workp.tile([P, BATCH * win], DT, tag="m")
                    sp2 = workp.tile([P, 16 * BATCH], DT, tag="sp2")
                    for j in range(bsz):
                        col = b0 + j * P
                        if DOUBLE_ROW:
                            nc.tensor.matmul(
                                spp[:, j * 16:j * 16 + 16],
                                lhsT=xi[:, :, col:col + P],
                                rhs=w8v,
                                start=True, stop=True,
                                perf_mode=mybir.MatmulPerfMode.DoubleRow)
                        else:
                            nc.tensor.matmul(spp[:, j * 16:j * 16 + 16],
                                             lhsT=xi[:, 0, col:col + P],
                                             rhs=w8v[:, 0, :],
                                             start=True, stop=False)
                            nc.tensor.matmul(spp[:, j * 16:j * 16 + 16],
                                             lhsT=xi[:, 1, col:col + P],
                                             rhs=w8v[:, 1, :],
                                             start=False, stop=True)
                    flush_pending()
                    # batched one-hot: M[i, b, w] = (rel[i, s+b] == w)
                    mv = m_t[:].rearrange("p (b w) -> p b w", w=win)
                    jv = jt[:, 0:bsz * win].rearrange("p (b w) -> p b w", w=win)
                    relb = cb_t[:, s:s + bsz].to_broadcast([P, bsz, win])
                    nc.vector.tensor_tensor(mv[:, 0:bsz, :], jv, relb,
                                            op=Alu.is_equal)
                    spv = spp[:].rearrange("p (b c) -> p b c", c=16)
                    sp2v = sp2[:].rearrange("p (b c) -> p b c", c=16)
                    nc.scalar.activation(sp2v[:, 0:bsz, 0:8], spv[:, 0:bsz, 0:8],
                                         Exp, scale=1.0 / sc_s)
                    nc.vector.tensor_tensor(sp2v[:, 0:bsz, 8:16],
                                            sp2v[:, 0:bsz, 0:8],
                                            spv[:, 0:bsz, 8:16], op=Alu.mult)
                    pending = (sp2, m_t, s, bsz, h)
                    s += bsz
                    b0 += bsz * P
            flush_pending()
            epilogue(1)
    nc.compile()
    return nc


# ---------------------------------------------------------------- entry point
def _invoke(tree_preds, group_ids, query, w_in, b_in, w_out, b_out, w_lin, b_lin,
            trace=False, **spmd_kwargs):
    import ml_dtypes
    np_dt = ml_dtypes.bfloat16
    x_dt = ml_dtypes.float8_e4m3

    tree_preds = np.asarray(tree_preds, dtype=np.float32)
    group_ids = np.asarray(group_ids, dtype=np.int32)

    W16, const = _fold_params(np.asarray(query), np.asarray(w_in), np.asarray(b_in),
                              np.asarray(w_out), np.asarray(b_out),
                              np.asarray(w_lin), np.asarray(b_lin))
    XT, RELT, woff, caps, nsub_half, win = _shard_prep(tree_preds, group_ids,
                                                       x_dt, np_dt)

    # power-of-2 prescales to land W16 in the fp8e4m3 sweet spot
    def _scale(block):
        m = float(np.abs(block).max())
        if m == 0.0:
            return 1.0
        return float(2.0 ** np.floor(np.log2(224.0 / m)))
    sc_s = _scale(W16[:, 0:8])
    sc_t = _scale(W16[:, 8:16])

    key = (tuple(caps), tuple(nsub_half), tuple(woff.tolist()), float(const), win,
           sc_s, sc_t, MACRO, LOOKAHEAD, DOUBLE_ROW)
    if _CACHE.get("key") != key:
        _CACHE["nc"] = _build_program(caps, nsub_half, woff, const, win, sc_s, sc_t)
        _CACHE["key"] = key
    nc = _CACHE["nc"]

    # w8: [P, 2, 16] -> flat [P, 32]; k-tile k holds E-half k
    Ws = W16.astype(np.float64)
    Ws[:, 0:8] *= sc_s
    Ws[:, 8:16] *= sc_t
    w8 = np.zeros((P, 2, 16), x_dt)
    w8[:, 0, :] = Ws[:P].astype(x_dt)
    w8[:, 1, :] = Ws[P:].astype(x_dt)
    w8 = w8.reshape(P, 32)

    in_maps = [{"xt": XT[c], "w8t": w8, "cbt": np.ascontiguousarray(RELT[c])}
               for c in range(NCORES)]

    from concourse.bass_utils import run_bass_kernel_spmd
    res = run_bass_kernel_spmd(nc, in_maps, core_ids=list(range(NCORES)),
                               trace=trace, **spmd_kwargs)

    out = np.empty((G, 1), np.float32)
    for c in range(NCORES):
        out[c * GC:(c + 1) * GC, 0] = res.results[c]["out"][0]
    return out, res


def kernel(tree_preds, group_ids, query, w_in, b_in, w_out, b_out, w_lin, b_lin):
    out, _ = _invoke(tree_preds, group_ids, query, w_in, b_in,
                     w_out, b_out, w_lin, b_lin)
    return out


# revision 8
# speedup vs baseline: 1.1434x; 1.1434x over previous
"""Trainium2 Bass kernel for AttentionAggregationNN (ragged single-query MHA pooling).

Algebraic reduction: with one shared query vector, softmax-attention pooling per
group collapses to, per instance i and head h:
    e[i,h]   = exp(s_h . x_i)          (softmax shift-invariance drops the
                                        constant logit term)
    val[i,h] = t_h . x_i
    out[g]   = sum_h (sum_{i in g} e*val) / (sum_{i in g} e) + CONST
where s_h = Wk_h^T q_h / sqrt(D), t_h = Wv_h^T (w_lin @ w_out)_h, folded on the
host in float64.

Device work per core (data-parallel over groups, host pre-sorts by group):
  mm1: SP[128,16] = X_subtile.T @ W16 in ONE fp8 DoubleRow matmul per subtile
       (contract dim 256 = 2 k-tiles of 128; X and W16 both fp8e4m3, W16
       prescaled by power-of-2 per column class, descaled via the ACT input
       scale for scores and the output tensor_scalar for vals)
  ACT: e = exp(scores * 1/sc_s);  DVE: ev = e * vals
  one-hot M[i,j] = (rel_gid[i] == j)              (one DVE op per batch)
  mm2: acc[16, win] += [e|ev].T @ M               (segment sums into PSUM,
                                                   3 PE column-group strips)
  epilogue per half: bf16 PE transpose of the accumulator, strip-sum via two
  DVE adds on [128,16], per-head reciprocal+mult on [128,8] (full DVE lanes),
  head-reduce, [128,1]->[1,128] transpose-matmul, fused *1/sc_t + CONST, and
  a per-half [1,128] output DMA (half0's rides under the stream).

DMA schedule: X is interleaved [P, 2, rows] in DRAM so ONE dma_start fetches
both E-halves of a macro-tile; macros alternate between the sync and scalar
rings.  This keeps the total hw-dma_start count low (the Tile scheduler
rotates just 8 completion semaphores — more dma_starts stall the rings on
semaphore reuse) while the ~2048-col macro size keeps the PE's wait
granularity fine enough to track the stream.
"""
import os

if os.environ.get("AXON_H4_ENABLED") == "1" or os.environ.get("AXON_TERMINAL_JOB_NAME"):
    plats = os.environ.get("JAX_PLATFORMS", "")
    if "axon" not in plats:
        os.environ["JAX_PLATFORMS"] = "axon,cpu"

import numpy as np

# ---------------------------------------------------------------- problem dims
N, G, E, H, D = 131072, 2048, 256, 8, 32
NCORES = 8
GC = G // NCORES        # 256 groups per core
HC = GC // 2            # 128 groups per half
P = 128                 # partition dim / subtile rows
BATCH = 16              # max subtiles per ACT/DVE batch
HALF_QUANT = 128        # row padding quantum per half (= P)
MACRO = 2048            # macro-tile cols
LOOKAHEAD = 4           # macros of DMA descriptor-gen lookahead

DOUBLE_ROW = False      # fp8 DoubleRow mm1: one matmul/subtile but the 2-k-tile
                        # LDWEIGHTS is slower than two plain fp8 loads (moving
                        # dim 16 can't amortize it) — measured 23us vs 16us PE

_CACHE: dict = {}


# ---------------------------------------------------------------- host algebra
def _fold_params(query, w_in, b_in, w_out, b_out, w_lin, b_lin):
    q64 = query.reshape(E).astype(np.float64)
    w64, b64 = w_in.astype(np.float64), b_in.astype(np.float64)
    wq, wk, wv = w64[:E], w64[E:2 * E], w64[2 * E:]
    bq, bk, bv = b64[:E], b64[E:2 * E], b64[2 * E:]
    q = wq @ q64 + bq
    qh = q.reshape(H, D)
    S = np.einsum("hde,hd->he", wk.reshape(H, D, E), qh) / np.sqrt(D)
    u = (w_lin.astype(np.float64) @ w_out.astype(np.float64)).reshape(E)
    uh = u.reshape(H, D)
    T = np.einsum("hde,hd->he", wv.reshape(H, D, E), uh)
    const = float(np.einsum("hd,hd->", uh, bv.reshape(H, D))
                  + w_lin.astype(np.float64).reshape(E) @ b_out.astype(np.float64)
                  + b_lin.astype(np.float64)[0])
    W16 = np.concatenate([S.T, T.T], axis=1)    # [E, 16]: cols 0:8 scores, 8:16 vals
    return W16, const


def _shard_prep(tree_preds, group_ids, x_dtype, np_dtype):
    sizes = np.bincount(group_ids, minlength=G)
    offsets = np.concatenate([[0], np.cumsum(sizes)]).astype(np.int64)
    sorter = np.argsort(group_ids, kind="stable")
    Xs = np.ascontiguousarray(tree_preds[sorter])
    gs = group_ids[sorter].astype(np.int64)

    # per (core, half): row range and size
    hstart = offsets[(np.arange(2 * NCORES) * HC)]
    hend = offsets[(np.arange(2 * NCORES) + 1) * HC]
    hrows = (hend - hstart).reshape(NCORES, 2)
    caps = [int(np.ceil(hrows[:, h].max() / HALF_QUANT) * HALF_QUANT) for h in (0, 1)]
    rows_cap = caps[0] + caps[1]
    nsub = rows_cap // P
    nsub_half = [caps[0] // P, caps[1] // P]

    # relative gid within half, -1 for pad rows
    grel = np.full((NCORES, rows_cap), -1, np.int64)
    for c in range(NCORES):
        for h in (0, 1):
            i = 2 * c + h
            n = hend[i] - hstart[i]
            col0 = h * caps[0]
            grel[c, col0:col0 + n] = gs[hstart[i]:hend[i]] - (c * GC + h * HC)
    gsub = grel.reshape(NCORES, nsub, P)
    lo = np.where(gsub >= 0, gsub, G).min(axis=(0, 2))
    hi = np.where(gsub >= 0, gsub, -1).max(axis=(0, 2))
    span = int(np.where(hi >= 0, hi - np.minimum(lo, hi) + 1, 1).max())
    win = 32 if span <= 28 else (64 if span <= 60 else 128)
    assert span <= win, f"one-hot span {span} > {win}"
    woff = np.minimum(np.where(lo < G, lo, 0), HC - win).astype(np.int64)
    assert ((hi < woff + win) | (hi < 0)).all()

    rel = np.where(gsub >= 0, gsub - woff[None, :, None], win).astype(np.float32)
    assert ((rel >= 0) & (rel <= win)).all()
    RELT = np.ascontiguousarray(rel.transpose(0, 2, 1)).astype(np_dtype)  # [NC,P,nsub]

    # interleaved X: XT[c, p, k, col] = E-half k of instance col (k-tiles for
    # the DoubleRow contraction)
    XT = np.zeros((NCORES, P, 2, rows_cap), x_dtype)
    for c in range(NCORES):
        for h in (0, 1):
            i = 2 * c + h
            n = hend[i] - hstart[i]
            col0 = h * caps[0]
            blk = Xs[hstart[i]:hend[i]].T.astype(x_dtype)
            XT[c, :, 0, col0:col0 + n] = blk[:P]
            XT[c, :, 1, col0:col0 + n] = blk[P:]
    return XT, RELT, woff, caps, nsub_half, win


def _macro_schedule(cap, first_half, last_half):
    """One dma_start per macro (both E-halves interleaved). First two macros
    of half0 small to prime the pipe quickly; last macro of half1 small to
    shorten the tail."""
    head = [512, 512] if first_half else []
    tail = [1024] if last_half else []
    mid = cap - sum(head) - sum(tail)
    assert mid >= 0, f"cap {cap} too small for schedule"
    sizes = head + [MACRO] * (mid // MACRO)
    if mid % MACRO:
        sizes.append(mid % MACRO)
    sizes += tail
    assert sum(sizes) == cap and all(s % HALF_QUANT == 0 for s in sizes)
    return sizes


# ---------------------------------------------------------------- bass program
def _build_program(caps, nsub_half, woff, const, win, sc_s, sc_t):
    import concourse.bass as bass
    import concourse.tile as tile
    from concourse import bacc, mybir

    DT = mybir.dt.bfloat16
    XDT = mybir.dt.float8e4
    F32 = mybir.dt.float32
    Exp = mybir.ActivationFunctionType.Exp
    Alu = mybir.AluOpType
    rows_cap = caps[0] + caps[1]
    nsub = nsub_half[0] + nsub_half[1]
    JW = BATCH * win

    nc = bacc.Bacc(None, target_bir_lowering=False)
    xt = nc.dram_tensor("xt", [P, 2, rows_cap], XDT, kind="ExternalInput")
    w8t = nc.dram_tensor("w8t", [P, 32], XDT, kind="ExternalInput")
    cbt = nc.dram_tensor("cbt", [P, nsub], DT, kind="ExternalInput")
    out = nc.dram_tensor("out", [1, GC], F32, kind="ExternalOutput")

    # macro schedule, flat across both halves; macro i rides ring i%2
    macros = []
    for h in (0, 1):
        m0 = 0 if h == 0 else caps[0]
        for msz in _macro_schedule(caps[h], first_half=(h == 0),
                                   last_half=(h == 1)):
            macros.append((h, m0, msz))
            m0 += msz
    n_macros = len(macros)

    with tile.TileContext(nc) as tc:
        with (
            tc.tile_pool(name="const", bufs=1) as constp,
            tc.tile_pool(name="xp", bufs=n_macros) as xp,
            tc.tile_pool(name="work", bufs=4) as workp,
            tc.tile_pool(name="ep", bufs=1) as epsb,
            tc.tile_pool(name="mm1", bufs=5, space="PSUM") as mm1p,
            tc.tile_pool(name="acc", bufs=1, space="PSUM") as accp,
            tc.tile_pool(name="tps", bufs=1, space="PSUM") as tpsp,
        ):
            xi_t = {}

            def issue(i):
                if i >= n_macros:
                    return
                _, im0, imsz = macros[i]
                xi_t[i] = xp.tile([P, 2, MACRO], XDT, tag="xi", name=f"xi_{i}")
                ring = nc.scalar if i % 2 == 0 else nc.sync
                ring.dma_start(xi_t[i][:, :, 0:imsz], xt[:, :, im0:im0 + imsz])

            # ---- ring priming: w8 on sync, macro0 on scalar, then relt and
            # the next few macros.
            w8 = constp.tile([P, 32], XDT)
            nc.sync.dma_start(w8[:], w8t[:])
            w8v = w8[:].rearrange("p (k c) -> p k c", c=16)
            issue(0)
            cb_t = constp.tile([P, nsub], DT)
            nc.scalar.dma_start(cb_t[:], cbt[:])
            for i in range(1, LOOKAHEAD):
                issue(i)

            # ---- device-generated constants (gpsimd/vector; rings stay free)
            # j-iota [P, JW]: value j % win, identical on every partition
            jt = constp.tile([P, JW], DT)
            nc.gpsimd.iota(jt[:], [[0, BATCH], [1, win]], channel_multiplier=0,
                           allow_small_or_imprecise_dtypes=True)
            # identity [P, P] bf16 for the PE transposes, via (row == col)
            pi_t = constp.tile([P, 1], F32)
            nc.gpsimd.iota(pi_t[:], [[0, 1]], channel_multiplier=1,
                           allow_small_or_imprecise_dtypes=True)
            jr_t = constp.tile([P, P], F32)
            nc.gpsimd.iota(jr_t[:], [[1, P]], channel_multiplier=0,
                           allow_small_or_imprecise_dtypes=True)
            ident = constp.tile([P, P], DT)
            nc.vector.tensor_tensor(ident[:], jr_t[:],
                                    pi_t[:].to_broadcast([P, P]),
                                    op=Alu.is_equal)
            # zeros for the PSUM-clearing matmul
            zw = constp.tile([P, 2 * HC], DT)
            nc.vector.memset(zw[:], 0.0)

            # Three accumulator strips per half at partition bases 0/32/64
            # (quadrant 3 is unusable per HW erratum): subtile s accumulates
            # into strip s%3; the inferred tile_position packs the mm2s into
            # disjoint PE column groups so they run concurrently.
            accB = accp.tile([P, 2 * HC], F32, tag="accB", name="accB")
            accs = [accB[:, 0:HC], accB[:, HC:2 * HC]]
            nc.tensor.matmul(accB[:, 0:2 * HC], lhsT=zw[:, 0:P], rhs=zw[:],
                             start=True, stop=False, skip_group_check=True)

            oo2 = epsb.tile([1, GC], F32, tag="oo2")

            def epilogue(h):
                # acc -> bf16 -> PE transpose: tpb[g, 32k+c]
                cc = epsb.tile([P, HC], DT, tag=f"cc{h}")
                nc.vector.tensor_copy(cc[:], accs[h])
                tpb = tpsp.tile([P, P], DT, tag="tpb")
                nc.tensor.transpose(tpb[:], cc[:], ident[:])
                # strip block to SBUF (DVE reads at most one PSUM input)
                ts_ = epsb.tile([P, 80], DT, tag=f"ts{h}")
                nc.vector.tensor_copy(ts_[:], tpb[:, 0:80])
                # strip-sum on the free axis: es[:, 0:8] e-sums, 8:16 ev-sums
                e01 = epsb.tile([P, 16], DT, tag=f"e01{h}")
                nc.vector.tensor_tensor(e01[:], ts_[:, 0:16], ts_[:, 32:48],
                                        op=Alu.add)
                es = epsb.tile([P, 16], DT, tag=f"es{h}")
                nc.vector.tensor_tensor(es[:], e01[:], ts_[:, 64:80], op=Alu.add)
                rec = epsb.tile([P, 8], F32, tag=f"rec{h}")
                nc.vector.reciprocal(rec[:], es[:, 0:8])
                rr = epsb.tile([P, 8], DT, tag=f"rr{h}")
                nc.vector.tensor_tensor(rr[:], rec[:], es[:, 8:16], op=Alu.mult)
                oo1 = epsb.tile([P, 1], DT, tag=f"oo1{h}")
                with nc.allow_low_precision("8-term head-sum; bf16 ample here"):
                    nc.vector.tensor_reduce(oo1[:], rr[:],
                                            axis=mybir.AxisListType.X, op=Alu.add)
                # [128,1] -> [1,128] via matmul with identity, then fused
                # val-descale + const into this half's slot of oo2, and a
                # per-half output DMA (half0's rides under the stream).
                opst = tpsp.tile([1, P], F32, tag="opst")
                nc.tensor.matmul(opst[:], lhsT=oo1[:], rhs=ident[:],
                                 start=True, stop=True, skip_group_check=True)
                nc.vector.tensor_scalar(oo2[0:1, h * HC:(h + 1) * HC],
                                        opst[:], 1.0 / sc_t, float(const),
                                        op0=Alu.mult, op1=Alu.add)
                nc.sync.dma_start(out[0:1, h * HC:(h + 1) * HC],
                                  oo2[0:1, h * HC:(h + 1) * HC])

            # ---- main loop. mm2s are emitted one batch late so the PE never
            # stalls on the ACT->DVE chain of the batch it just produced.
            pending = None          # (sp2, m_t, s0, bsz, h)

            ranges = [(0, nsub_half[0]), (nsub_half[0], nsub_half[0] + nsub_half[1])]
            last_q = [{k: max(s for s in range(b, e) if s % 3 == k)
                       for k in range(3)} for (b, e) in ranges]

            def flush_pending():
                nonlocal pending
                if pending is None:
                    return
                sp2, m_t, s0, bsz, ph = pending
                acc = accs[ph]
                for j in range(bsz):
                    s_i = s0 + j
                    k = s_i % 3
                    nc.tensor.matmul(
                        acc[32 * k:32 * k + 16, woff[s_i]:woff[s_i] + win],
                        lhsT=sp2[:, j * 16:j * 16 + 16],
                        rhs=m_t[:, j * win:(j + 1) * win],
                        start=False, stop=(s_i == last_q[ph][k]),
                        skip_group_check=True)
                pending = None

            s = 0
            cur_h = 0
            for mi, (h, m0, msz) in enumerate(macros):
                if h != cur_h:
                    flush_pending()
                    epilogue(cur_h)
                    cur_h = h
                issue(mi + LOOKAHEAD)
                xi = xi_t.pop(mi)
                b0 = 0
                while b0 < msz:
                    bsz = min(BATCH, (msz - b0) // P)      # subtiles in batch
                    spp = mm1p.tile([P, 16 * BATCH], F32)
                    m_t = workp.tile([P, BATCH * win], DT, tag="m")
                    sp2 = workp.tile([P, 16 * BATCH], DT, tag="sp2")
                    for j in range(bsz):
                        col = b0 + j * P
                        if DOUBLE_ROW:
                            nc.tensor.matmul(
                                spp[:, j * 16:j * 16 + 16],
                                lhsT=xi[:, :, col:col + P],
                                rhs=w8v,
                                start=True, stop=True,
                                perf_mode=mybir.MatmulPerfMode.DoubleRow)
                        else:
                            nc.tensor.matmul(spp[:, j * 16:j * 16 + 16],
                                             lhsT=xi[:, 0, col:col + P],
                                             rhs=w8v[:, 0, :],
                                             start=True, stop=False)
                            nc.tensor.matmul(spp[:, j * 16:j * 16 + 16],
                                             lhsT=xi[:, 1, col:col + P],
                                             rhs=w8v[:, 1, :],
                                             start=False, stop=True)
                    flush_pending()
                    # batched one-hot: M[i, b, w] = (rel[i, s+b] == w)
                    mv = m_t[:].rearrange("p (b w) -> p b w", w=win)
                    jv = jt[:, 0:bsz * win].rearrange("p (b w) -> p b w", w=win)
                    relb = cb_t[:, s:s + bsz].to_broadcast([P, bsz, win])
                    nc.vector.tensor_tensor(mv[:, 0:bsz, :], jv, relb,
                                            op=Alu.is_equal)
                    spv = spp[:].rearrange("p (b c) -> p b c", c=16)
                    sp2v = sp2[:].rearrange("p (b c) -> p b c", c=16)
                    nc.scalar.activation(sp2v[:, 0:bsz, 0:8], spv[:, 0:bsz, 0:8],
                                         Exp, scale=1.0 / sc_s)
                    nc.vector.tensor_tensor(sp2v[:, 0:bsz, 8:16],
                                            sp2v[:, 0:bsz, 0:8],
                                            spv[:, 0:bsz, 8:16], op=Alu.mult)
                    pending = (sp2, m_t, s, bsz, h)
                    s += bsz
                    b0 += bsz * P
            flush_pending()
            epilogue(1)
    nc.compile()
    return nc


# ---------------------------------------------------------------- entry point
def _invoke(tree_preds, group_ids, query, w_in, b_in, w_out, b_out, w_lin, b_lin,
            trace=False, **spmd_kwargs):
    import ml_dtypes
    np_dt = ml_dtypes.bfloat16
    x_dt = ml_dtypes.float8_e4m3

    tree_preds = np.asarray(tree_preds, dtype=np.float32)
    group_ids = np.asarray(group_ids, dtype=np.int32)

    W16, const = _fold_params(np.asarray(query), np.asarray(w_in), np.asarray(b_in),
                              np.asarray(w_out), np.asarray(b_out),
                              np.asarray(w_lin), np.asarray(b_lin))
    XT, RELT, woff, caps, nsub_half, win = _shard_prep(tree_preds, group_ids,
                                                       x_dt, np_dt)

    # power-of-2 prescales to land W16 in the fp8e4m3 sweet spot
    def _scale(block):
        m = float(np.abs(block).max())
        if m == 0.0:
            return 1.0
        return float(2.0 ** np.floor(np.log2(224.0 / m)))
    sc_s = _scale(W16[:, 0:8])
    sc_t = _scale(W16[:, 8:16])

    key = (tuple(caps), tuple(nsub_half), tuple(woff.tolist()), float(const), win,
           sc_s, sc_t, MACRO, LOOKAHEAD, DOUBLE_ROW)
    if _CACHE.get("key") != key:
        _CACHE["nc"] = _build_program(caps, nsub_half, woff, const, win, sc_s, sc_t)
        _CACHE["key"] = key
    nc = _CACHE["nc"]

    # w8: [P, 2, 16] -> flat [P, 32]; k-tile k holds E-half k
    Ws = W16.astype(np.float64)
    Ws[:, 0:8] *= sc_s
    Ws[:, 8:16] *= sc_t
    w8 = np.zeros((P, 2, 16), x_dt)
    w8[:, 0, :] = Ws[:P].astype(x_dt)
    w8[:, 1, :] = Ws[P:].astype(x_dt)
    w8 = w8.reshape(P, 32)

    in_maps = [{"xt": XT[c], "w8t": w8, "cbt": np.ascontiguousarray(RELT[c])}
               for c in range(NCORES)]

    from concourse.bass_utils import run_bass_kernel_spmd
    res = run_bass_kernel_spmd(nc, in_maps, core_ids=list(range(NCORES)),
                               trace=trace, **spmd_kwargs)

    out = np.empty((G, 1), np.float32)
    for c in range(NCORES):
        out[c * GC:(c + 1) * GC, 0] = res.results[c]["out"][0]
    return out, res


def kernel(tree_preds, group_ids, query, w_in, b_in, w_out, b_out, w_lin, b_lin):
    out, _ = _invoke(tree_preds, group_ids, query, w_in, b_in,
                     w_out, b_out, w_lin, b_lin)
    return out
